# revision 1
# baseline (speedup 1.0000x reference)
"""Trainium2 Bass kernel for causal multi-head attention (B=2, T=2048, C=1024, H=16, D=64).

Sharding (8 NeuronCores): data-parallel over the 2 batches x tensor-parallel over
4 head-groups (4 heads each). Core c handles batch c//4, heads 4*(c%4)..4*(c%4)+3.
Each core computes its 4 heads' QKV projections, causal attention, and a partial
output projection against its slice of Wo's columns; the host sums the 4 partials
per batch (the row-parallel unshard).

Device algorithm (per core), all matmuls bf16 with fp32 PSUM accumulation:
  QT = WqS @ x^T            [256, T]   (d-major; head pair per 128-row block)
  KT = WkS @ x^T            [256, T]
  V  = x @ WvS^T            [T, 4, 66] (t-major, head-strided, ones column at 64)
  per head-pair hk, q-tile i (512 wide), key-tile j (128 wide, j <= 4i+3):
    S^T both heads -> one [128, 2, 512] PSUM tile via two row-group-packed
      matmuls (K=64 each, tile_position (0,0) and (64,0) -> concurrent on PE)
    P^T = exp(S^T / 8)      (one ACT op per pair tile; unsafe softmax)
    P^T *= tri-mask         (diagonal blocks; mask broadcast over the head dim)
    O^T_h[:, i] += V_h[j]^T @ P^T_h   (M=65: ones column accumulates l)
  per head-pair: recip = exp(-ln(l)), broadcast, scale O^T  (overlaps attention)
  Y_partial = O^T-as-lhsT @ WoS^T    [T, 1024] fp32 -> DRAM
"""

import numpy as np

B, T, C = 2, 2048, 1024
H, D = 16, 64
HPC = 4  # heads per core
N_CORES = 8
DH = HPC * D  # 256: per-core projection width

_compiled = None


def _patch_act_tables():
    """Make Exp and Ln resolve to the one table set that holds both
    (natural_log_exp_and_others), so interleaved softmax exps and the
    ln/exp reciprocal never thrash ACT_TABLE_LOADs."""
    import functools

    import concourse.hw_specs as hw_specs
    import concourse.mybir as mybir
    from concourse import bacc

    if getattr(bacc, "_act_tables_patched", False):
        return
    orig = hw_specs.get_activation_tables

    @functools.cache
    def patched(arch):
        tabs = {k: set(v) for k, v in orig(arch).items()}
        E = mybir.ActivationFunctionType.Exp
        L = mybir.ActivationFunctionType.Ln
        keep = "natural_log_exp_and_others"
        if keep in tabs and E in tabs[keep] and L in tabs[keep]:
            for name, fns in tabs.items():
                if name != keep:
                    fns.discard(E)
                    fns.discard(L)
        return tabs

    bacc.get_activation_tables = patched
    bacc._act_tables_patched = True


def _build():
    import concourse.bass as bass
    import concourse.mybir as mybir
    from concourse import bacc
    from concourse.tile import TileContext

    _patch_act_tables()

    dt = mybir.dt
    BF = dt.bfloat16
    F32 = dt.float32
    ts = bass.ts
    Act = mybir.ActivationFunctionType

    P = 128
    NQ = T // 512   # 4 q-tiles of 512
    NK = T // 128   # 16 key-tiles of 128
    KC = C // 128   # 8 contraction subtiles for the projections

    nc = bacc.Bacc("TRN2", target_bir_lowering=False, debug=False)

    xT_d = nc.dram_tensor("xT", [C, T], BF, kind="ExternalInput")
    wq_d = nc.dram_tensor("wqT", [C, DH], BF, kind="ExternalInput")
    wk_d = nc.dram_tensor("wkT", [C, DH], BF, kind="ExternalInput")
    wv_d = nc.dram_tensor("wvT", [C, DH], BF, kind="ExternalInput")
    wo_d = nc.dram_tensor("woT", [DH, C], BF, kind="ExternalInput")
    y_d = nc.dram_tensor("y", [T, C], F32, kind="ExternalOutput")

    xT_r = xT_d[:].rearrange("(ko p) (nq t) -> p ko nq t", p=P, nq=NQ)
    wq_r = wq_d[:].rearrange("(ko p) m -> p ko m", p=P)
    wk_r = wk_d[:].rearrange("(ko p) m -> p ko m", p=P)
    wv_r = wv_d[:].rearrange("(ko p) m -> p ko m", p=P)
    wo_r = wo_d[:].rearrange("(ko p) n -> p ko n", p=P)

    with TileContext(nc) as tc:
        with (
            tc.tile_pool(name="persist", bufs=1) as persist,
            tc.tile_pool(name="ptiles", bufs=20) as ptiles,
            tc.tile_pool(name="ytiles", bufs=4) as ytiles,
            tc.tile_pool(name="ltmp", bufs=2) as ltmp,
            tc.tile_pool(name="psum_s", bufs=2, space="PSUM") as psum_s,
            tc.tile_pool(name="psum_o", bufs=2, space="PSUM") as psum_o,
            tc.tile_pool(name="psum_p", bufs=2, space="PSUM") as psum_p,
        ):
            # ---- persistent SBUF tensors -------------------------------
            x_sb = persist.tile([P, KC, T], BF, tag="x")          # x^T
            wq_sb = persist.tile([P, KC, DH], BF, tag="wq")
            wk_sb = persist.tile([P, KC, DH], BF, tag="wk")
            wv_sb = persist.tile([P, KC, DH], BF, tag="wv")
            wo_sb = persist.tile([P, DH // P, C], BF, tag="wo")
            qT_sb = persist.tile([P, DH // P, T], BF, tag="qT")
            kT_sb = persist.tile([P, DH // P, T], BF, tag="kT")
            v_sb = persist.tile([P, NK, HPC, 66], BF, tag="v")
            oT_sb = persist.tile([P, DH // P, T], BF, tag="oT")
            cmask = persist.tile([P, 2, P], BF, tag="cmask")
            l_sb = persist.tile([2, HPC // 2, T], F32, tag="l")
            rec_sb = persist.tile([2, HPC // 2, T], BF, tag="rec")

            # ---- small weight DMAs first (the projections need them) ----
            nc.sync.dma_start(wq_sb[:], wq_r)
            nc.sync.dma_start(wk_sb[:], wk_r)
            nc.sync.dma_start(wv_sb[:], wv_r)
            # x in half-row chunks, first halves (q-tiles 0-1) of every ko
            # first: the opening projection k-loop only needs those, so the
            # first PSUM tile completes ~2MB sooner
            for nh in range(2):
                for ko in range(KC):
                    nc.sync.dma_start(
                        x_sb[:, ko, ts(nh, T // 2)], xT_r[:, ko, 2 * nh : 2 * nh + 2, :]
                    )
            nc.sync.dma_start(wo_sb[:], wo_r)

            # ---- constants: causal corner mask + V's ones column -------
            # diagonal [128,128] corner: keep 1.0 where col >= row, else 0.0
            nc.gpsimd.memset(v_sb[:, :, :, 64:66], 1.0)
            nc.gpsimd.memset(cmask[:], 1.0)
            for half in range(2):
                nc.gpsimd.affine_select(
                    out=cmask[:, half, :],
                    in_=cmask[:, half, :],
                    compare_op=mybir.AluOpType.is_ge,
                    fill=0.0,
                    base=0,
                    pattern=[[1, P]],
                    channel_multiplier=-1,
                )

            # ---- QT / KT projections ----------------------------------
            # n-pairs share one weight load (lhsT constant across the pair);
            # m-order q0,k0,q1,k1 so head-pair 0 scores can start early
            for m in range(DH // P):
                for w_sb, out_sb in ((wq_sb, qT_sb), (wk_sb, kT_sb)):
                    for np2 in range(NQ // 2):
                        ps0 = psum_p.tile([P, 512], F32, tag="pp")
                        ps1 = psum_p.tile([P, 512], F32, tag="pp")
                        pss = [ps0, ps1]
                        for k in range(KC):
                            for half in range(2):
                                nc.tensor.matmul(
                                    pss[half][:],
                                    w_sb[:, k, ts(m, P)],
                                    x_sb[:, k, ts(2 * np2 + half, 512)],
                                    start=(k == 0),
                                    stop=(k == KC - 1),
                                )
                        for half in range(2):
                            nc.vector.tensor_copy(
                                out_sb[:, m, ts(2 * np2 + half, 512)], pss[half][:]
                            )

            # ---- V projection: out[t-tile, head, d] --------------------
            for mt in range(NK):
                ps_full = psum_p.tile([P, 512], F32, tag="pp")
                ps = ps_full[:, :DH]
                for k in range(KC):
                    nc.tensor.matmul(
                        ps[:],
                        x_sb[:, k, ts(mt, P)],
                        wv_sb[:, k, :],
                        start=(k == 0),
                        stop=(k == KC - 1),
                    )
                nc.vector.tensor_copy(
                    v_sb[:, mt, :, 0:64], ps.rearrange("p (h d) -> p h d", d=64)
                )

            # ---- attention: head pairs (2*hk, 2*hk+1) ------------------
            # normalize multiplies are deferred one (hk, i) iteration so the
            # DVE never head-of-line blocks on the recip/broadcast chain
            pending_mults = []

            def flush_mults():
                for args in pending_mults:
                    nc.vector.tensor_mul(*args)
                pending_mults.clear()

            deferred_recips = []

            def emit_recip(hk, i):
                lnl = ltmp.tile([2, 512], F32, tag="lnl")
                nc.scalar.activation(lnl[:], l_sb[:, hk, ts(i, 512)], Act.Ln)
                nc.scalar.activation(
                    rec_sb[:, hk, ts(i, 512)], lnl[:], Act.Exp, scale=-1.0
                )
                for half in range(2):
                    rec0 = ltmp.tile([1, 512], BF, tag="rec0")
                    nc.gpsimd.dma_start(
                        rec0[:], rec_sb[half : half + 1, hk, ts(i, 512)]
                    )
                    rb = ltmp.tile([P, 512], BF, tag="rb")
                    nc.gpsimd.partition_broadcast(rb[:], rec0[0:1, :])
                    hp = 64 * half
                    pending_mults.append(
                        (
                            oT_sb[hp : hp + 64, hk, ts(i, 512)],
                            oT_sb[hp : hp + 64, hk, ts(i, 512)],
                            rb[hp : hp + 64, :],
                        )
                    )

            for hk in range(DH // P):
                for i in range(NQ):
                    prev = list(pending_mults)
                    pending_mults.clear()
                    jmax = 4 * i + 3
                    pts = []
                    for j in range(jmax + 1):
                        # diagonal tiles with offset t: columns < 128t are
                        # fully causal-masked, so skip computing them
                        c0 = P * (j - 4 * i) if j >= 4 * i else 0
                        sp = psum_s.tile([P, 2, 512], F32, tag="s")
                        # row-group-packed pair: head 2*hk in PE rows 0-63,
                        # head 2*hk+1 in rows 64-127 -> concurrent matmuls
                        for half in range(2):
                            hp = 64 * half
                            nc.tensor.matmul(
                                sp[:, half, c0:],
                                kT_sb[hp : hp + 64, hk, ts(j, P)],
                                qT_sb[hp : hp + 64, hk, 512 * i + c0 : 512 * (i + 1)],
                                start=True,
                                stop=True,
                                tile_position=(hp, 0),
                            )
                        pt = ptiles.tile([P, 2, 512], BF, tag="p")
                        if j >= 4 * i:
                            t = j - 4 * i
                            # cols < 128t are never computed nor read: the
                            # scores, exp, and AV all operate on cols >= 128t
                            nc.scalar.activation(
                                pt[:, :, P * t :], sp[:, :, P * t :],
                                Act.Exp, scale=0.125,
                            )
                            nc.vector.tensor_mul(
                                pt[:, :, P * t : P * (t + 1)],
                                pt[:, :, P * t : P * (t + 1)],
                                cmask[:],
                            )
                        else:
                            nc.scalar.activation(pt[:], sp[:], Act.Exp, scale=0.125)
                        pts.append(pt)
                    for args in prev:
                        nc.vector.tensor_mul(*args)
                    op0 = psum_o.tile([P, 512], F32, tag="o")
                    op1 = psum_o.tile([P, 512], F32, tag="o")
                    ops = [op0, op1]
                    for j in range(jmax + 1):
                        # diagonal tile t only contributes to columns >= 128t
                        # (pt is zero below); j=0 is always the full-width
                        # start=True writer, so partial-width accumulates are
                        # safe for every element
                        c0 = P * (j - 4 * i) if j >= 4 * i else 0
                        for half in range(2):
                            h = 2 * hk + half
                            nc.tensor.matmul(
                                ops[half][0:65, c0:],
                                v_sb[:, j, h, 0:65],
                                pts[j][:, half, c0:],
                                start=(j == 0),
                                stop=(j == jmax),
                            )
                    for half in range(2):
                        h = 2 * hk + half
                        hp = 64 * half
                        nc.vector.tensor_copy(
                            oT_sb[hp : hp + 64, hk, ts(i, 512)], ops[half][0:64, :]
                        )
                        lt = ltmp.tile([P, 512], F32, tag="lt")
                        nc.vector.tensor_copy(lt[64:65, :], ops[half][64:65, :])
                        nc.gpsimd.dma_start(
                            l_sb[half : half + 1, hk, ts(i, 512)], lt[64:65, :]
                        )

                    if i == NQ - 1:
                        # the last q-tile's recip chain would sit unresolved in
                        # the ACT queue right at the pair boundary; its Y
                        # consumers run at the tail anyway, so defer it
                        deferred_recips.append((hk, i))
                    else:
                        emit_recip(hk, i)

            for hk_i in deferred_recips:
                emit_recip(*hk_i)
                flush_mults()
            flush_mults()

            # ---- output projection: Y = O @ WoS^T ----------------------
            # kc outside n so each O^T weight load serves two matmuls
            for mt in range(NK):
                py0 = psum_p.tile([P, 512], F32, tag="pp")
                py1 = psum_p.tile([P, 512], F32, tag="pp")
                pss = [py0, py1]
                for kc in range(DH // P):
                    for n in range(C // 512):
                        nc.tensor.matmul(
                            pss[n][:],
                            oT_sb[:, kc, ts(mt, P)],
                            wo_sb[:, kc, ts(n, 512)],
                            start=(kc == 0),
                            stop=(kc == DH // P - 1),
                        )
                for n in range(C // 512):
                    yt = ytiles.tile([P, 512], F32, tag="y")
                    # explicit DVE: nc.any would route these to ACT, competing
                    # with the softmax exps that pace the attention pipeline
                    nc.vector.tensor_copy(yt[:], pss[n][:])
                    nc.sync.dma_start(y_d[ts(mt, P), ts(n, 512)], yt[:])

    nc.compile()
    return nc


def _get_compiled():
    global _compiled
    if _compiled is None:
        _compiled = _build()
    return _compiled


def make_inputs(x, Wq, Wk, Wv, Wo):
    """Shard the full inputs into the 8 per-core input maps (host-side prep)."""
    import ml_dtypes

    bf16 = ml_dtypes.bfloat16
    x = np.asarray(x)
    in_maps = []
    for c in range(N_CORES):
        b, g = divmod(c, HPC)
        rows = slice(g * DH, (g + 1) * DH)
        in_maps.append(
            {
                "xT": np.ascontiguousarray(x[b].T).astype(bf16),
                "wqT": np.ascontiguousarray(np.asarray(Wq)[rows, :].T).astype(bf16),
                "wkT": np.ascontiguousarray(np.asarray(Wk)[rows, :].T).astype(bf16),
                "wvT": np.ascontiguousarray(np.asarray(Wv)[rows, :].T).astype(bf16),
                "woT": np.ascontiguousarray(np.asarray(Wo)[:, rows].T).astype(bf16),
            }
        )
    return in_maps


def assemble(results):
    """Sum the 4 tensor-parallel partials per batch into the full output."""
    y = np.zeros((B, T, C), dtype=np.float32)
    for c in range(N_CORES):
        b = c // HPC
        y[b] += results[c]["y"]
    return y


def kernel(x, Wq, Wk, Wv, Wo):
    from concourse.bass_utils import run_bass_kernel_spmd

    nc = _get_compiled()
    in_maps = make_inputs(x, Wq, Wk, Wv, Wo)
    res = run_bass_kernel_spmd(nc, in_maps, list(range(N_CORES)))
    return assemble(res.results)

